# revision 32
# baseline (speedup 1.0000x reference)
"""VQ codebook (K-means batch) loss kernel for 8 Trainium2 NeuronCores.

loss = mean((quantize(x) - x)^2)
     = (sum(x^2) + sum_rows min_k(||w_k||^2 - 2 x.w_k)) / (N*D)

Sharding: data-parallel over flattened N (4096 rows/core), codebook replicated.

Device strategy (per core), shaped by two hardware rules the BIR verifier
enforces (GPSIMD cannot touch PSUM; any vector op may read at most ONE
non-scalar input from PSUM):
  - PE: fp8(e4m3) matmuls in DoubleRow perf mode (2 contraction k-tiles per
    instruction -> 0.5 cycles/row) produce -2 x.w into [128, 2x512] PSUM
    tiles.  32 row tiles per core.
  - ||w_k||^2 enters PSUM via a cheap fp8 DoubleRow "ones" matmul per tile
    (residual-quantized rows with scales 4/2/1, error <0.15).
  - The PSUM drain (the bottleneck: each of the 4096x1024 distances must pass
    through DVE or ACT at ~1 elem/cycle) is split across both engines:
      * DVE tiles: one tensor_reduce(min) over the whole [128, 2, 512] PSUM
        tile -> exact row mins.
      * ACT tiles: one Exp activation with accum_out computes
        sum_k exp((c - d)/T); the host finishes the softmin
        min ~= c - T*ln(sum)  (error ~1e-4 relative, tol is 2e-2).
  - PE is pre-warmed with dummy matmuls so real work runs at full clock; the
    first loads are split fine-grained so the drain engines start ASAP.
  - All inputs are host-prepared fp8/fp32 and loaded via HWDGE on SP.
Host computes the exact sum(x^2) term (input prep, 0.1% of the FLOPs) and
combines everything in fp64.
"""

import numpy as np
import ml_dtypes
from contextlib import ExitStack
import os as _os

import concourse.bass as bass
import concourse.tile as tile
from concourse import bacc, mybir
from concourse.bass_utils import run_bass_kernel_spmd

N_CORES = 8
D = 512
K = 1024
R_TOT = 64 * 512
R = R_TOT // N_CORES          # 4096 rows per core
NT = R // 128                 # 32 row tiles
GROUPS = 4                    # x load groups
TPG = NT // GROUPS            # 8 tiles per group
RG = R // GROUPS              # 1024 rows per group

BIG = 3.0e38
SOFT_T = 2.0
SOFT_C = 290.0
F8 = mybir.dt.float8e4
NPF8 = ml_dtypes.float8_e4m3
BF16 = mybir.dt.bfloat16
FP32 = mybir.dt.float32

WARM = int(_os.environ.get("KWARM", "8"))
RED_MODE = _os.environ.get("KRED", "mix")   # mix | dve
NDVE = int(_os.environ.get("KNDVE", "16"))  # tiles on the DVE ttr path
PBUFS = int(_os.environ.get("KPBUFS", "4"))

_CACHE = {}


def _dve_tiles():
    if RED_MODE == "dve":
        return set(range(NT))
    # Bresenham spread of NDVE DVE tiles across NT; tile 0 stays ACT so the
    # first drain does not wait on the wsq-broadcast load.
    s = {m for m in range(NT) if (m * NDVE) // NT != ((m + 1) * NDVE) // NT}
    if 0 in s and NDVE < NT:
        s.discard(0)
        for m in range(NT):
            if m not in s:
                s.add(m)
                break
    return s


def _build():
    if "nc" in _CACHE:
        return _CACHE["nc"]
    nc = bacc.Bacc(
        "TRN2",
        target_bir_lowering=False,
        debug=False,
        enable_asserts=False,
        num_devices=N_CORES,
    )
    xd = nc.dram_tensor("xd", [128, GROUPS, 2, 2, RG], F8, kind="ExternalInput").ap()
    w8 = nc.dram_tensor("w8", [128, 2, 2, K], F8, kind="ExternalInput").ap()
    # wsq fp8 residual rows (cols 0..K) and the 4/2/1 scale columns (K..K+128)
    wq = nc.dram_tensor("wq", [2, 2, K + 128], F8, kind="ExternalInput").ap()
    out_o = nc.dram_tensor("outs", [128, 2, NT], FP32, kind="ExternalOutput").ap()

    dve_set = _dve_tiles()

    with tile.TileContext(nc) as tc, ExitStack() as ctx:
        wpool = ctx.enter_context(tc.tile_pool(name="w", bufs=1))
        xdpool = ctx.enter_context(tc.tile_pool(name="xd", bufs=3))
        scrpool = ctx.enter_context(tc.tile_pool(name="scr", bufs=2))
        opool = ctx.enter_context(tc.tile_pool(name="outs", bufs=1))
        ppool = ctx.enter_context(tc.tile_pool(name="ps", bufs=PBUFS, space="PSUM"))

        ones2 = wpool.tile([2, 128], BF16)
        wq_s = wpool.tile([2, 2, K + 128], F8)
        w_s = wpool.tile([128, 2, 2, K], F8)
        bias_s = wpool.tile([128, 1], FP32)
        out_s = opool.tile([128, 2, NT], FP32)

        nc.gpsimd.memset(ones2[:], 1.0)
        nc.gpsimd.memset(bias_s[:], SOFT_C / SOFT_T)
        nc.gpsimd.memset(out_s[:, :, :], 1.0)
        # explicit load order on the SP sequencer controls DMA-device order;
        # each dma_start costs ~650ns of issue pipeline, so keep loads coarse.
        nc.sync.dma_start(out=wq_s[:, :, :], in_=wq[:, :, :])

        xdt = {}

        def load_xd(g, pr=None, rows=None):
            if g not in xdt:
                xdt[g] = xdpool.tile([128, 2, 2, RG], F8, tag="xd", name=f"xd{g}")
            t = xdt[g]
            if pr is None:
                nc.sync.dma_start(out=t[:, :, :, :], in_=xd[:, g, :, :, :])
            elif rows is None:
                nc.sync.dma_start(out=t[:, pr, :, :], in_=xd[:, g, pr, :, :])
            else:
                nc.sync.dma_start(
                    out=t[:, pr, :, rows[0]:rows[1]],
                    in_=xd[:, g, pr, :, rows[0]:rows[1]],
                )

        # first row-tiles' data as early as possible: k-half 0 of the codebook
        # plus the first 256 rows let tiles 0-1 run (their drains are split by
        # k-half below, so the drain engines start before w8 fully lands)
        nc.sync.dma_start(out=w_s[:, :, :, 0:512], in_=w8[:, :, :, 0:512])
        xdt[0] = xdpool.tile([128, 2, 2, RG], F8, tag="xd", name="xd0")
        nc.sync.dma_start(out=xdt[0][:, :, :, 0:256], in_=xd[:, 0, :, :, 0:256])
        nc.sync.dma_start(out=w_s[:, :, :, 512:K], in_=w8[:, :, :, 512:K])
        nc.sync.dma_start(out=xdt[0][:, :, :, 256:RG], in_=xd[:, 0, :, :, 256:RG])
        load_xd(1)

        # PE warmup: junk matmuls (need only ones2) start the clock ramp
        # while the first loads land; the group is closed by stop=True.
        pw = ppool.tile([128, 2, 512], FP32, tag="ps", name="warm")
        for i in range(WARM):
            nc.tensor.matmul(
                pw[:, 0, 0:128], lhsT=ones2[:], rhs=ones2[:],
                start=(i == 0), stop=(i == WARM - 1),
            )

        for g in range(GROUPS):
            xg = xdt[g]
            for t_ in range(TPG):
                m = g * TPG + t_
                is_dve = m in dve_set
                split = m < 2  # drain each k-half separately during pipe fill
                ps = ppool.tile([128, 2, 512], FP32, tag="ps", name=f"ps{m}")

                def mm_half(h):
                    # wsq -> PSUM via fp8 residual rows (scales 4/2/1)
                    nc.tensor.matmul(
                        ps[:, h, :], lhsT=wq_s[:, :, K:K + 128],
                        rhs=wq_s[:, :, h * 512:(h + 1) * 512],
                        start=True, stop=False,
                        perf_mode=mybir.MatmulPerfMode.DoubleRow,
                    )
                    for pr in range(2):
                        nc.tensor.matmul(
                            ps[:, h, :],
                            lhsT=xg[:, pr, :, t_ * 128:(t_ + 1) * 128],
                            rhs=w_s[:, pr, :, h * 512:(h + 1) * 512],
                            start=False,
                            stop=(pr == 1),
                            perf_mode=mybir.MatmulPerfMode.DoubleRow,
                        )

                def drain(src, half, slot, col):
                    if is_dve:
                        nc.vector.tensor_reduce(
                            out=out_s[:, slot, col:col + 1], in_=src,
                            axis=(mybir.AxisListType.X if half is not None
                                  else mybir.AxisListType.XY),
                            op=mybir.AluOpType.min,
                        )
                    else:
                        scr = scrpool.tile(
                            [128, 2, 512], BF16, tag="scr", name=f"scr{m}_{slot}"
                        )
                        out_ap = scr[:, half, :] if half is not None else scr[:, :, :]
                        nc.scalar.activation(
                            out=out_ap, in_=src,
                            func=mybir.ActivationFunctionType.Exp,
                            scale=-1.0 / SOFT_T, bias=bias_s[:],
                            accum_out=out_s[:, slot, col:col + 1],
                        )

                if split:
                    mm_half(0)
                    drain(ps[:, 0, :], 0, 0, m)
                    mm_half(1)
                    drain(ps[:, 1, :], 1, 1, m)
                else:
                    mm_half(0)
                    mm_half(1)
                    drain(ps[:, :, :], None, 0 if is_dve else 1, m)
            if g + 2 < GROUPS:
                load_xd(g + 2)
            if g == GROUPS - 2:
                # flush the first chunk of the outputs while compute continues
                nc.sync.dma_start(out=out_o[:, :, 0:16], in_=out_s[:, :, 0:16])

        nc.sync.dma_start(out=out_o[:, :, 16:NT], in_=out_s[:, :, 16:NT])

    nc.compile()
    _CACHE["nc"] = nc
    return nc


def _prep(inputs, weight):
    x = np.asarray(inputs, dtype=np.float32).reshape(-1, D)  # [32768, 512]
    w = np.asarray(weight, dtype=np.float32)                 # [1024, 512]

    w8f = (-2.0 * w.T).astype(NPF8)                          # [512, 1024]
    # d = pr*256 + j*128 + p  ->  [p, pr, j, k]
    w8prep = np.ascontiguousarray(
        w8f.reshape(2, 2, 128, K).transpose(2, 0, 1, 3)
    )                                                        # [128, 2, 2, K]
    wsq = (w.astype(np.float64) ** 2).sum(axis=1)            # exact
    # fp8 residual rows r0,r1,r2 with scales 4,2,1 (slot (1,1) is zero)
    r0 = (wsq / 4).astype(NPF8)
    r1 = ((wsq - 4 * r0.astype(np.float64)) / 2).astype(NPF8)
    r2 = (wsq - 4 * r0.astype(np.float64) - 2 * r1.astype(np.float64)).astype(NPF8)
    wq = np.zeros((2, 2, K + 128), dtype=NPF8)
    wq[0, 0, :K] = r0
    wq[0, 1, :K] = r1
    wq[1, 0, :K] = r2
    wq[0, 0, K:] = 4.0
    wq[0, 1, K:] = 2.0
    wq[1, 0, K:] = 1.0

    in_maps = []
    for cidx in range(N_CORES):
        sh = x[cidx * R:(cidx + 1) * R]                      # [4096, 512]
        x8 = sh.astype(NPF8)                                 # [R, D]
        # [p, g, pr, j, r']  with d = pr*256 + j*128 + p, row = g*RG + r'
        xdprep = np.ascontiguousarray(
            x8.reshape(GROUPS, RG, 2, 2, 128).transpose(4, 0, 2, 3, 1)
        )                                                    # [128, G, 2, 2, RG]
        in_maps.append({"xd": xdprep, "w8": w8prep, "wq": wq})
    return in_maps


def _run(inputs, weight, trace=False, **kw):
    nc = _build()
    in_maps = _prep(inputs, weight)
    res = run_bass_kernel_spmd(nc, in_maps, list(range(N_CORES)), trace=trace, **kw)
    x = np.asarray(inputs, dtype=np.float64).reshape(-1, D)
    total = float((x * x).sum())
    dve_set = _dve_tiles()
    for r in res.results:
        o = r["outs"].astype(np.float64)
        for m in range(NT):
            if m < 2:  # k-half-split tiles: one half per slot
                if m in dve_set:
                    total += np.minimum(o[:, 0, m], o[:, 1, m]).sum()
                else:
                    total += (SOFT_C - SOFT_T * np.log(o[:, 0, m] + o[:, 1, m])).sum()
            elif m in dve_set:
                total += o[:, 0, m].sum()
            else:
                total += (SOFT_C - SOFT_T * np.log(o[:, 1, m])).sum()
    loss = total / (R_TOT * D)
    return np.array(loss, dtype=np.float32), res


def kernel(inputs, weight):
    return _run(inputs, weight)[0]
